# revision 74
# baseline (speedup 1.0000x reference)
"""Causal self-attention (B=4, T=2048, C=1024, H=16, D=64) on 8 TRN2 cores.

Sharding: core c handles batch b=c//2, head-group g=c%2 (8 heads, 512 of the
1024 qkv feature columns). Fully SPMD, no collectives: each core computes a
partial out-projection; the host sums the two partials per batch, adds bp,
and gathers k/v.

Per-core pipeline:
  A) q/k/v = xT.T @ W in float32r (TF32-like, full PE rate at N=512); bias
     via a K=1 ones-matmul; RoPE on DVE reading PSUM directly (4 ops per
     tensor-chunk); PE-transpose q,k -> qT,kT [feat, t] (bf16); v -> v_aug
     [s, 8*(64+1)] bf16 with a ones column per head (softmax denominator
     comes free as PV row 64).
  B) per (t-block j of 512, head h): S^T[s,t] in bf16 (kT stationary,
     K=64), exp on ScalarE (psum -> sbuf bf16, scale=1/8 folded in, no
     max-subtraction needed at this score scale); causal structure via
     block skipping + a 128-wide "stair" over the 4 diagonal s-chunks
     (grouped into one [128,768] + one [128,512] exp with fused host-mask
     multiplies on DVE); PV accumulates [65, 512]; normalize
     = DVE reciprocal of the denominator row + gpsimd partition_broadcast
     + DVE multiply -> yT bf16.
  C) out-proj psum += yT.T @ WpT (bf16) -> partial out (f32).

Emission is software-pipelined: transposes lag one chunk so the PE queue
never head-of-line blocks on RoPE; PV lags scores by one pair so PE streams
ahead of the ACT exp; A-chunks 4..15 and out-proj blocks are drip-fed
through phase B to fill ACT-bound PE gaps.
"""
import numpy as np
import ml_dtypes

B, T, C = 4, 2048, 1024
H, D = 16, 64
HLOC = H // 2          # heads per core
FS = HLOC * D          # 512 feature cols per core
NKC = C // 128         # 8 contraction chunks
NTC = T // 128         # 16 t-chunks
NTB = T // 512         # 4 t-blocks
SCALE = 1.0 / np.sqrt(D).astype(np.float32)
ROPE_BASE = 10000.0

_COMPILED = None


def _round_fp32r(x):
    u = np.ascontiguousarray(x, dtype=np.float32).view(np.uint32)
    r = (u.astype(np.uint64) + 0x7FF + ((u >> 12) & 1)) & 0xFFFFF000
    return r.astype(np.uint32).view(np.float32)


def _build():
    import concourse.tile as tile
    import concourse.mybir as mybir
    import concourse.bass as bass
    from concourse import bacc

    F32 = mybir.dt.float32
    F32R = mybir.dt.float32r
    BF16 = mybir.dt.bfloat16
    EXP = mybir.ActivationFunctionType.Exp

    def rep_mid(ap2d, reps):
        # [P, n] -> [P, reps, n] via a stride-0 middle dim
        return bass.AP(
            tensor=ap2d.tensor, offset=ap2d.offset,
            ap=[list(ap2d.ap[0]), [0, reps], list(ap2d.ap[1])],
        )

    nc = bacc.Bacc("TRN2", target_bir_lowering=False, debug=False)

    xT_d = nc.declare_dram_parameter("xT", [C, T], F32R, isOutput=False)
    wq_d = nc.declare_dram_parameter("wq", [C, FS], F32R, isOutput=False)
    wk_d = nc.declare_dram_parameter("wk", [C, FS], F32R, isOutput=False)
    wv_d = nc.declare_dram_parameter("wv", [C, FS], F32R, isOutput=False)
    bq_d = nc.declare_dram_parameter("bq", [1, FS], F32R, isOutput=False)
    bk_d = nc.declare_dram_parameter("bk", [1, FS], F32R, isOutput=False)
    bv_d = nc.declare_dram_parameter("bv", [1, FS], F32R, isOutput=False)
    wp_d = nc.declare_dram_parameter("wp", [FS, C], BF16, isOutput=False)
    cos_d = nc.declare_dram_parameter("cosw", [128, NTC, 64], F32, isOutput=False)
    sin_d = nc.declare_dram_parameter("sinw", [128, NTC, 64], F32, isOutput=False)
    ident_d = nc.declare_dram_parameter("ident", [128, 128], F32R, isOutput=False)
    cst_d = nc.declare_dram_parameter("cst", [128, 2], F32R, isOutput=False)
    msk_d = nc.declare_dram_parameter("msk", [128, 3, 1024], BF16, isOutput=False)

    k_out = nc.declare_dram_parameter("k_nat", [T, FS], F32, isOutput=True)
    v_out = nc.declare_dram_parameter("v_nat", [T, FS], F32, isOutput=True)
    o_out = nc.declare_dram_parameter("o_part", [T, C], F32, isOutput=True)

    with tile.TileContext(nc) as tc:
        with (
            tc.tile_pool(name="const", bufs=1) as const,
            tc.tile_pool(name="io", bufs=2) as io,
            tc.tile_pool(name="work", bufs=3) as work,
            tc.tile_pool(name="workB", bufs=3) as workB,
            tc.tile_pool(name="pmain", bufs=3, space="PSUM") as pmain,
            tc.tile_pool(name="pS", bufs=2, space="PSUM") as pSp,
            tc.tile_pool(name="pacc", bufs=1, space="PSUM") as pacc,
        ):
            # ---- resident constants (critical-path-first DMA order) ----
            xt0 = io.tile([128, NKC, 128], F32R, tag="xt")
            nc.sync.dma_start(
                out=xt0, in_=xT_d[:, 0:128].rearrange("(n p) t -> p n t", p=128)
            )
            bq = const.tile([1, FS], F32R)
            bk = const.tile([1, FS], F32R)
            bv = const.tile([1, FS], F32R)
            nc.sync.dma_start(out=bq, in_=bq_d[:, :])
            nc.sync.dma_start(out=bk, in_=bk_d[:, :])
            nc.sync.dma_start(out=bv, in_=bv_d[:, :])
            cosw = const.tile([128, NTC, 64], F32)
            sinw = const.tile([128, NTC, 64], F32)
            nc.sync.dma_start(out=cosw, in_=cos_d[:, :, :])
            nc.sync.dma_start(out=sinw, in_=sin_d[:, :, :])
            wq = const.tile([128, NKC, FS], F32R)
            wk = const.tile([128, NKC, FS], F32R)
            wv = const.tile([128, NKC, FS], F32R)
            # per-K-chunk DMAs, in consumption order (all wq, then wk, wv)
            for w_t, w_dr in ((wq, wq_d), (wk, wk_d), (wv, wv_d)):
                for kk in range(NKC):
                    nc.sync.dma_start(
                        out=w_t[:, kk, :], in_=w_dr[kk * 128:(kk + 1) * 128, :]
                    )
            ident = const.tile([128, 128], F32R)
            nc.sync.dma_start(out=ident, in_=ident_d[:, :])
            ident_b = const.tile([128, 128], BF16)
            nc.vector.tensor_copy(out=ident_b[:, :], in_=ident[:, :].bitcast(F32))
            cst = const.tile([128, 2], F32R)
            nc.sync.dma_start(out=cst, in_=cst_d[:, :])
            ones_row = cst[0:1, 0:1]
            wp = const.tile([128, FS // 128, C], BF16)
            msk = const.tile([128, 3, 1024], BF16)

            # ---- resident activations ----
            qT = const.tile([128, FS // 128, T], BF16)   # [f, t] 4 f-chunks
            kT = const.tile([128, FS // 128, T], BF16)
            v_aug = const.tile([128, NTC, HLOC * (D + 1)], BF16)

            # ================= phase A: projections + rope + transpose ===
            # Software-pipelined: transposes for chunk i-1 are emitted between
            # chunk i's projection matmuls so the PE queue never waits on rope.
            def emit_trans(rot, dst, tsl, on_act=True):
                pt = pSp.tile([128, FS], rot.dtype, tag="pS")
                idn = ident[:, :].bitcast(rot.dtype) if rot.dtype == F32R else ident_b[:, :]
                for fc in range(FS // 128):
                    nc.tensor.transpose(
                        pt[:, fc * 128:(fc + 1) * 128],
                        rot[:, fc * 128:(fc + 1) * 128], idn,
                    )
                src = pt[:, :].rearrange("p (fc t) -> p fc t", t=128)
                if on_act:
                    nc.scalar.copy(out=dst[:, :, tsl], in_=src)
                else:
                    nc.vector.tensor_copy(out=dst[:, :, tsl], in_=src)

            pending = []  # (rot, dst, tsl) transposes from the previous chunk

            def emit_A(i):
                tsl = slice(i * 128, (i + 1) * 128)
                if i == 0:
                    xt = xt0
                else:
                    xt = io.tile([128, NKC, 128], F32R, tag="xt")
                    nc.sync.dma_start(
                        out=xt, in_=xT_d[:, tsl].rearrange("(n p) t -> p n t", p=128)
                    )
                for name, w_t, b_t in (("q", wq, bq), ("k", wk, bk), ("v", wv, bv)):
                    ps = pmain.tile([128, FS], F32, tag="pmain")
                    for kk in range(NKC):
                        nc.tensor.matmul(
                            ps[:, :], xt[:, kk, :], w_t[:, kk, :],
                            start=(kk == 0), stop=False,
                        )
                    nc.tensor.matmul(
                        ps[:, :], ones_row.broadcast_to((1, 128)), b_t[:, :],
                        start=False, stop=True,
                    )
                    if pending:
                        emit_trans(*pending.pop(0), on_act=(i < 5))
                    if name == "v":
                        sb = work.tile([128, FS], F32, tag="sb_v")
                        nc.scalar.copy(out=sb[:, :], in_=ps[:, :])
                        # v_aug: per head 64 cols of v then a ones column
                        va = v_aug[:, i, :].rearrange("p (h e) -> p h e", e=D + 1)
                        nc.gpsimd.tensor_copy(
                            out=va[:, :, 0:D],
                            in_=sb[:, :].rearrange("p (h d) -> p h d", d=D),
                        )
                        nc.gpsimd.tensor_copy(
                            out=va[:, :, D], in_=cst[:, 0:1].broadcast_to((128, HLOC))
                        )
                        nc.sync.dma_start(out=v_out[tsl, :], in_=sb[:, :])
                    else:
                        # rope (4 ops, reading psum directly):
                        # mc = x*cos, ms = x*sin; even = mc_e - ms_o ; odd = ms_e + mc_o
                        mc = work.tile([128, HLOC, D], F32, tag="mc")
                        ms = work.tile([128, HLOC, D], F32, tag="ms")
                        psh = ps[:, :].rearrange("p (h e) -> p h e", e=D)
                        nc.vector.tensor_mul(mc[:, :, :], psh, rep_mid(cosw[:, i, :], HLOC))
                        nc.vector.tensor_mul(ms[:, :, :], psh, rep_mid(sinw[:, i, :], HLOC))
                        mc3 = mc[:, :, :].rearrange("p h (i two) -> p h i two", two=2)
                        ms3 = ms[:, :, :].rearrange("p h (i two) -> p h i two", two=2)
                        rot = work.tile(
                            [128, FS], BF16 if name == "q" else F32R, tag=f"rot_{name}"
                        )
                        rot3 = rot[:, :].rearrange(
                            "p (h i two) -> p h i two", h=HLOC, two=2
                        )
                        nc.vector.tensor_sub(rot3[:, :, :, 0], mc3[:, :, :, 0], ms3[:, :, :, 1])
                        nc.vector.tensor_add(rot3[:, :, :, 1], ms3[:, :, :, 0], mc3[:, :, :, 1])
                        if name == "k":
                            nc.sync.dma_start(
                                out=k_out[tsl, :], in_=rot[:, :].bitcast(F32)
                            )
                        pending.append((rot, qT if name == "q" else kT, tsl))

            for i in range(4):
                emit_A(i)
            # deferred bulk loads (first needed in phase B/C)
            nc.sync.dma_start(out=msk, in_=msk_d[:, :, :])
            nc.sync.dma_start(out=wp, in_=wp_d.rearrange("(n p) f -> p n f", p=128))

            # ================= phase B/C: attention + out-proj per t-block =
            # Out-proj matmuls for block j-1 are drip-fed between block j's
            # heads so PE has filler work during ACT-bound stretches.
            va3 = v_aug[:, :, :].rearrange("p n (h e) -> p n h e", e=D + 1)

            def make_oproj(yT, j, m, nb):
                def emit(on_act=False):
                    tsl = slice(j * 512 + m * 128, j * 512 + (m + 1) * 128)
                    po_ = pmain.tile([128, 512], F32, tag="pmain")
                    for kc in range(FS // 128):
                        nc.tensor.matmul(
                            po_[:, :],
                            yT[:, kc, m * 128:(m + 1) * 128],
                            wp[:, kc, nb * 512:(nb + 1) * 512],
                            start=(kc == 0), stop=(kc == FS // 128 - 1),
                        )
                    ob = work.tile([128, 512], F32, tag="ob")
                    if on_act:
                        nc.scalar.copy(out=ob[:, :], in_=po_[:, :])
                    else:
                        nc.vector.tensor_copy(out=ob[:, :], in_=po_[:, :])
                    nc.sync.dma_start(
                        out=o_out[tsl, nb * 512:(nb + 1) * 512], in_=ob[:, :]
                    )
                return emit

            oproj_pending = []
            a_left = list(range(4, NTC))
            for j in range(NTB):
                yT = io.tile([128, FS // 128, 512], BF16, tag="yT")
                for h in range(HLOC):
                    if h % 2 == 1 and a_left:
                        emit_A(a_left.pop(0))
                    if oproj_pending:
                        oproj_pending.pop(0)()
                    fc, po = h // 2, 64 * (h % 2)
                    psl = slice(po, po + D)
                    py = pacc.tile([D + 1, 512], F32, tag="py")
                    # full s-chunk pairs below the diagonal band (no mask),
                    # then a 128-wide "stair" over the 4 diagonal s-chunks.
                    npair = 2 * j  # full pairs cover s-chunks 0 .. 4j-1

                    def emit_pv(pT, i0, py=py, h=h):
                        for c in range(2):
                            nc.tensor.matmul(
                                py[:, :],
                                va3[:, i0 + c, h, :],
                                pT[:, c * 512:(c + 1) * 512],
                                start=(i0 + c == 0), stop=False,
                                skip_group_check=True,
                            )

                    pv_pending = None  # software-pipeline PV one pair behind
                    for p_ in range(npair):
                        i0 = 2 * p_
                        pS = pSp.tile([128, 1024], F32, tag="pS")
                        for c in range(2):
                            nc.tensor.matmul(
                                pS[:, c * 512:(c + 1) * 512],
                                kT[psl, fc, (i0 + c) * 128:(i0 + c + 1) * 128],
                                qT[psl, fc, j * 512:(j + 1) * 512],
                                start=True, stop=True,
                            )
                        if pv_pending is not None:
                            emit_pv(*pv_pending)
                        pT = workB.tile([128, 1024], BF16, tag="pT")
                        nc.scalar.activation(
                            out=pT[:, :], in_=pS[:, :], func=EXP, scale=float(SCALE)
                        )
                        pv_pending = (pT, i0)

                    # diagonal band as 4 "strips": s-chunk 4j+idx covers the
                    # contiguous t-range [idx*128, 512) of this block. Strips
                    # 0,3 share psum tile A (cols 0:512 | 512:640); strips 1,2
                    # share tile B (0:384 | 384:640). One exp + one fused
                    # host-mask multiply per tile; one PV matmul per strip.
                    pSA = pSp.tile([128, 1024], F32, tag="pS")
                    nc.tensor.matmul(
                        pSA[:, 0:512],
                        kT[psl, fc, (4 * j) * 128:(4 * j + 1) * 128],
                        qT[psl, fc, j * 512:(j + 1) * 512],
                        start=True, stop=True,
                    )
                    nc.tensor.matmul(
                        pSA[:, 512:640],
                        kT[psl, fc, (4 * j + 3) * 128:(4 * j + 4) * 128],
                        qT[psl, fc, j * 512 + 384:(j + 1) * 512],
                        start=True, stop=True,
                    )
                    # strip2 lives in tile A cols [640:896] (entirely bank 1)
                    nc.tensor.matmul(
                        pSA[:, 640:896],
                        kT[psl, fc, (4 * j + 2) * 128:(4 * j + 3) * 128],
                        qT[psl, fc, j * 512 + 256:(j + 1) * 512],
                        start=True, stop=True,
                    )
                    pSB = pSp.tile([128, 1024], F32, tag="pS")
                    nc.tensor.matmul(
                        pSB[:, 0:384],
                        kT[psl, fc, (4 * j + 1) * 128:(4 * j + 2) * 128],
                        qT[psl, fc, j * 512 + 128:(j + 1) * 512],
                        start=True, stop=True,
                    )
                    if pv_pending is not None:
                        emit_pv(*pv_pending)
                        pv_pending = None
                    pTA = workB.tile([128, 896], BF16, tag="pT2")
                    nc.scalar.activation(
                        out=pTA[:, :], in_=pSA[:, :896], func=EXP,
                        scale=float(SCALE),
                    )
                    nc.vector.tensor_mul(pTA[:, :], pTA[:, :], msk[:, 0, 0:896])
                    nc.tensor.matmul(
                        py[:, 0:512], va3[:, 4 * j, h, :], pTA[:, 0:512],
                        start=(j == 0), stop=False, skip_group_check=True,
                    )
                    pTB = workB.tile([128, 384], BF16, tag="pT3")
                    nc.scalar.activation(
                        out=pTB[:, :], in_=pSB[:, :384], func=EXP,
                        scale=float(SCALE),
                    )
                    nc.vector.tensor_mul(pTB[:, :], pTB[:, :], msk[:, 1, 0:384])
                    nc.tensor.matmul(
                        py[:, 384:512], va3[:, 4 * j + 3, h, :], pTA[:, 512:640],
                        start=False, stop=False, skip_group_check=True,
                    )
                    nc.tensor.matmul(
                        py[:, 256:512], va3[:, 4 * j + 2, h, :], pTA[:, 640:896],
                        start=False, stop=False, skip_group_check=True,
                    )
                    nc.tensor.matmul(
                        py[:, 128:512], va3[:, 4 * j + 1, h, :], pTB[:, 0:384],
                        start=False, stop=True, skip_group_check=True,
                    )
                    rrow = work.tile([1, 512], F32, tag="rrow")
                    with nc.allow_low_precision(reason="softmax recip"):
                        nc.vector.reciprocal(out=rrow[:, :], in_=py[D:D + 1, :])
                    rb = work.tile([D, 512], F32, tag="rb")
                    nc.gpsimd.partition_broadcast(rb[:, :], rrow[:, :])
                    nc.vector.tensor_mul(yT[psl, fc, :], py[0:D, :], rb[:, :])
                oproj_pending.extend(
                    make_oproj(yT, j, m, nb)
                    for m in range(4) for nb in range(C // 512)
                )
                if not a_left and pending:
                    while pending:
                        emit_trans(*pending.pop(0))
            for emit in oproj_pending:
                emit(on_act=True)

    nc.finalize()
    return nc


def _get_compiled():
    global _COMPILED
    if _COMPILED is None:
        _COMPILED = _build()
    return _COMPILED


def _host_inputs(x, Wq, bq, Wk, bk, Wv, bv, Wp, bp):
    pos = np.arange(T, dtype=np.float64)
    inv = 1.0 / (ROPE_BASE ** (np.arange(0, D, 2, dtype=np.float64) / D))
    fr = pos[:, None] * inv[None, :]                  # [T, 32]
    cosw = np.repeat(np.cos(fr).astype(np.float32), 2, axis=1)   # [T, 64]
    sinw = np.repeat(np.sin(fr).astype(np.float32), 2, axis=1)
    cosw = cosw.reshape(NTC, 128, 64).transpose(1, 0, 2)
    sinw = sinw.reshape(NTC, 128, 64).transpose(1, 0, 2)
    ident = _round_fp32r(np.eye(128, dtype=np.float32))
    cst = np.ones((128, 2), dtype=np.float32)
    ss = np.arange(128)[:, None, None]
    cc = np.arange(2)[None, :, None]
    tt = np.arange(512)[None, None, :]
    tri128 = (np.arange(128)[None, :] >= np.arange(128)[:, None])
    msk = np.ones((128, 3, 1024))
    # strip-tile masks: triangle at each strip's first (diagonal) t-chunk.
    # tile A = [strip0 (t 0:512) | strip3 (t 384:512) | strip2 (t 256:512)]:
    # triangles at cols 0:128, 512:640, 640:768
    for col in (0, 512, 640):
        msk[:, 0, col:col + 128] = tri128
    # tile B = [strip1 (t 128:512)]: triangle at cols 0:128
    msk[:, 1, 0:128] = tri128
    msk[:, 2, :128] = tri128
    msk = msk.astype(ml_dtypes.bfloat16)

    in_maps = []
    for c in range(8):
        b, g = c // 2, c % 2
        cols = slice(g * FS, (g + 1) * FS)
        in_maps.append({
            "xT": _round_fp32r(np.ascontiguousarray(x[b].T)),
            "wq": _round_fp32r(np.ascontiguousarray(Wq[cols, :].T)),
            "wk": _round_fp32r(np.ascontiguousarray(Wk[cols, :].T)),
            "wv": _round_fp32r(np.ascontiguousarray(Wv[cols, :].T)),
            "bq": _round_fp32r(bq[cols][None, :]),
            "bk": _round_fp32r(bk[cols][None, :]),
            "bv": _round_fp32r(bv[cols][None, :]),
            "wp": np.ascontiguousarray(Wp[:, cols].T).astype(ml_dtypes.bfloat16),
            "cosw": np.ascontiguousarray(cosw),
            "sinw": np.ascontiguousarray(sinw),
            "ident": ident,
            "cst": cst,
            "msk": msk,
        })
    return in_maps


def kernel(x, Wq, bq, Wk, bk, Wv, bv, Wp, bp, _trace=False):
    from concourse.bass_utils import run_bass_kernel_spmd

    x = np.asarray(x, dtype=np.float32)
    nc = _get_compiled()
    in_maps = _host_inputs(x, np.asarray(Wq), np.asarray(bq), np.asarray(Wk),
                           np.asarray(bk), np.asarray(Wv), np.asarray(bv),
                           np.asarray(Wp), np.asarray(bp))
    res = run_bass_kernel_spmd(nc, in_maps, list(range(8)), trace=_trace)

    out = np.empty((B, T, C), dtype=np.float32)
    k = np.empty((B, H, T, D), dtype=np.float32)
    v = np.empty((B, H, T, D), dtype=np.float32)
    bp32 = np.asarray(bp, dtype=np.float32)
    for c in range(8):
        b, g = c // 2, c % 2
        r = res.results[c]
        if g == 0:
            out[b] = r["o_part"]
        else:
            out[b] += r["o_part"]
        k[b, g * HLOC:(g + 1) * HLOC] = (
            r["k_nat"].reshape(T, HLOC, D).transpose(1, 0, 2)
        )
        v[b, g * HLOC:(g + 1) * HLOC] = (
            r["v_nat"].reshape(T, HLOC, D).transpose(1, 0, 2)
        )
    out += bp32[None, None, :]
    if _trace:
        return (out, k, v), res
    return out, k, v
